# revision 60
# baseline (speedup 1.0000x reference)
"""3-layer GAT (nn_GATfly3) on 8 TRN2 NeuronCores.

Strategy (v3)
-------------
Nodes are sharded across the 8 cores (1000 each, padded to 1024 slots =
8 chunks x 128); each core owns its nodes' incident edges (by dst).
All inputs ship as ONE packed int32 blob per core (device-side APs
bitcast/reshape slices of it) to minimize per-dispatch marshaling.
Per layer:
  1. every core computes payload rows [feat(512)|a_s|a_d|pad] in BF16
     for its nodes and all-gathers them into a Shared DRAM table
     [8192 x 640] (row = 1280B, dma_gather needs %256B),
  2. edge phase: one dma_gather per layer pulls every edge's dst
     attention columns (256B slices); per 128-dst chunk a dma_gather
     pulls the src rows (bf16, 1280B each); softmax weights
     p = exp(leaky_relu(a_s+a_d)) are computed per edge (no per-segment
     max: |e| stays small enough for fp32 exp; the reference's
     max-subtraction cancels out of alpha up to the 1e-16 epsilon) and
     broadcast-multiplied into static 0/1 dst-indicator tiles S_T;
     a single PSUM accumulation per chunk then yields both the weighted
     message sum and the softmax denominator,
  3. the layer weight is applied AFTER the segment sum (linearity:
     segsum(alpha*(x@W)[src]) == segsum(alpha*x[src]) @ W), so gathers
     stay 512-wide even for the 3-head layer 3; matmuls/transposes run
     in bf16,
  4. pre-norm activations stay resident in SBUF (bf16) across the tiny
     graph-norm stats AllReduce; gamma/beta application is skipped when
     they are identity (rebuilds if not); pooling is one indicator
     matmul + AllReduce; the tiny MLP runs replicated on every core.
Layer-1 attention dots fold into the x@W1 matmul (x @ (W1@attv));
layer-2/3 payload dots run on the PE (transpose + matmul with attT).
"""

import math
import os

import numpy as np

import concourse.mybir as mybir
import concourse.tile as tile
from concourse import bacc
from concourse.bass_utils import run_bass_kernel_spmd
from concourse.masks import make_identity

dt = mybir.dt
AF = mybir.ActivationFunctionType
OP = mybir.AluOpType

# problem constants
N, E, B = 8000, 80000, 16
IN, HID, HEADS, OUT = 1025, 512, 3, 256
EPS_LN, EPS_SM, SLOPE = 1e-5, 1e-16, 0.2

NCORES = 8
NLOC = N // NCORES          # 1000 owned nodes per core
CHUNKS = 8                  # dst chunks per core
NSLOT = CHUNKS * 128        # 1024 node slots per core
GROWS = NCORES * NSLOT      # 8192 rows in the gathered payload table
INP = 1152                  # IN padded to 9*128
PW = 640                    # payload row width (bf16 -> 1280B, %256B)
AW = 128                    # attention column block (256B in bf16)


# ----------------------------------------------------------------------------
# host-side preprocessing
# ----------------------------------------------------------------------------

_DTSZ = {"int16": 2, "bf16": 2, "f32": 4, "f32r": 4}


def _blob_layout(KT):
    """Single packed input blob: name -> (byte offset, shape, dtype tag).
    512B-aligned entries; identical on host (packing) and device (APs)."""
    ESL = KT * 128
    specs = [
        ("sidx", (128, CHUNKS, ESL // 16), "int16"),
        ("didx", (128, CHUNKS * ESL // 16), "int16"),
        ("st", (CHUNKS, 128, KT, 128), "bf16"),
        ("xt", (CHUNKS, INP // 128, 128, 128), "bf16"),
        ("segm", (128, CHUNKS, B), "f32r"),
        ("segmT", (B, CHUNKS, 128), "f32r"),
        ("w1p", (INP, HID), "bf16"),
        ("w1av", (INP, 2), "bf16"),
        ("w2", (HID, HID), "bf16"),
        ("w3", (HID, HEADS * HID), "bf16"),
        ("gbbrep", (128, 3, 3, HEADS * HID), "f32"),
        ("attT", (2, 4, 128, 6), "bf16"),
        ("rw", (HEADS * HID, HID), "f32"),
        ("l1w", (HID, HID // 2), "f32"),
        ("l2w", (HID // 2, HID // 4), "f32"),
        ("ow", (HID // 4, OUT), "f32"),
        ("rbp", (128, 4), "f32"),
        ("l1bp", (128, 2), "f32"),
        ("l2bp", (128, 1), "f32"),
        ("obp", (128, 2), "f32"),
        ("cinv", (B, 4), "f32"),
    ]
    lay = {}
    off = 0
    for nm, shape, d in specs:
        nbytes = int(np.prod(shape)) * _DTSZ[d]
        lay[nm] = (off, shape, d)
        off += (nbytes + 511) // 512 * 512
    return lay, off


def _pack_blob(m, KT):
    lay, total = _blob_layout(KT)
    u8 = np.zeros(total, np.uint8)
    for nm, (off, shape, d) in lay.items():
        arr = np.ascontiguousarray(m[nm])
        b = arr.tobytes()
        assert len(b) == int(np.prod(shape)) * _DTSZ[d], \
            f"{nm}: {len(b)} != {shape} x {d}"
        u8[off:off + len(b)] = np.frombuffer(b, np.uint8)
    return u8.view(np.int32)


def _wrap_idx(idx, nslots):
    """Edge-slot indices -> dma_gather SBUF layout [128, nslots//16].

    Index i is read from partition i%16, column i//16; the 16-row block is
    replicated across the 8 GPSIMD core groups (partitions 0..127)."""
    assert len(idx) == nslots and nslots % 16 == 0
    w = np.zeros((16, nslots // 16), np.int16)
    w[np.arange(nslots) % 16, np.arange(nslots) // 16] = idx.astype(np.int16)
    return np.tile(w, (8, 1))


def preprocess(inputs):
    import ml_dtypes
    bf = ml_dtypes.bfloat16
    x = np.asarray(inputs["x"], np.float32)
    ei = np.asarray(inputs["edge_index"], np.int64)
    batch = np.asarray(inputs["batch"], np.int64)
    src_g, dst_g = ei[0], ei[1]

    # ---- per-core node -> slot assignment (edge-balanced chunks) ----
    deg = np.bincount(dst_g, minlength=N)
    slot_of_node = np.full(N, -1, np.int64)
    node_of_slot = [np.full(NSLOT, -1, np.int64) for _ in range(NCORES)]
    for k in range(NCORES):
        nodes = np.arange(k * NLOC, (k + 1) * NLOC)
        order = nodes[np.argsort(-deg[nodes], kind="stable")]
        load = np.zeros(CHUNKS, np.int64)
        fill = np.zeros(CHUNKS, np.int64)
        for n in order:
            cands = np.where(fill < 128)[0]
            c = cands[np.argmin(load[cands])]
            slot_of_node[n] = c * 128 + fill[c]
            node_of_slot[k][c * 128 + fill[c]] = n
            fill[c] += 1
            load[c] += deg[n]
    core_of_node = np.arange(N) // NLOC
    grow_of_node = core_of_node * NSLOT + slot_of_node

    # ---- per-core edge lists grouped by (chunk, dst slot) ----
    KT = 0
    per_core_edges = []
    for k in range(NCORES):
        mask = core_of_node[dst_g] == k
        es, ed = src_g[mask], dst_g[mask]
        dslot = slot_of_node[ed]
        order = np.argsort(dslot, kind="stable")
        es, dslot = es[order], dslot[order]
        chunk = dslot // 128
        counts = np.bincount(chunk, minlength=CHUNKS)
        KT = max(KT, int(math.ceil(counts.max() / 128)))
        per_core_edges.append((es, dslot, chunk, counts))

    ESL = KT * 128  # edge slots per chunk

    cnt = np.bincount(batch, minlength=B).astype(np.float64)
    cntc = np.maximum(cnt, 1.0)
    cinv = np.zeros((B, 4), np.float32)
    cinv[:, 0] = 1.0 / (cntc * HID)
    cinv[:, 1] = 1.0 / (cntc * (HEADS * HID))
    cinv[:, 2] = 1.0 / cntc

    per_core = []
    for k in range(NCORES):
        es, dslot, chunk, counts = per_core_edges[k]
        st = np.zeros((CHUNKS, KT, 128, 128), np.float32)
        src_rows = np.zeros((CHUNKS, ESL), np.int64)
        dst_rows = np.zeros(CHUNKS * ESL, np.int64)
        for c in range(CHUNKS):
            sel = chunk == c
            ec = int(counts[c])
            s_c, d_c = es[sel], dslot[sel] - c * 128
            j = np.arange(ec)
            st[c, j // 128, j % 128, d_c] = 1.0
            src_rows[c, :ec] = grow_of_node[s_c]
            dst_rows[c * ESL: c * ESL + ec] = k * NSLOT + c * 128 + d_c
        sidx = np.stack([_wrap_idx(src_rows[c], ESL) for c in range(CHUNKS)], axis=1)
        didx = _wrap_idx(dst_rows, CHUNKS * ESL)
        # device layout [c, partition(edge), ktile, dst], bf16 (0/1 exact)
        st = np.ascontiguousarray(np.transpose(st, (0, 2, 1, 3)))
        st = st.astype(bf)

        xt = np.zeros((CHUNKS, INP // 128, 128, 128), np.float32)
        xpad = np.zeros((NSLOT, INP), np.float32)
        valid = node_of_slot[k] >= 0
        xpad[valid, :IN] = x[node_of_slot[k][valid]]
        xr = xpad.reshape(CHUNKS, 128, INP // 128, 128)
        xt[:] = np.transpose(xr, (0, 2, 3, 1))

        segm = np.zeros((CHUNKS, 128, B), np.float32)
        segmT = np.zeros((CHUNKS, B, 128), np.float32)
        bslot = np.full(NSLOT, -1, np.int64)
        bslot[valid] = batch[node_of_slot[k][valid]]
        for c in range(CHUNKS):
            for p in range(128):
                g = bslot[c * 128 + p]
                if g >= 0:
                    segm[c, p, g] = 1.0
                    segmT[c, g, p] = 1.0

        per_core.append(dict(
            sidx=sidx, didx=didx, st=st, xt=xt.astype(bf),
            segm=np.ascontiguousarray(np.transpose(segm, (1, 0, 2))),
            segmT=np.ascontiguousarray(np.transpose(segmT, (1, 0, 2)))))

    # ---- shared (replicated) weights ----
    w1 = np.asarray(inputs["w1"], np.float32)
    w2 = np.asarray(inputs["w2"], np.float32)
    w3 = np.asarray(inputs["w3"], np.float32)
    w1p = np.zeros((INP, HID), np.float32)
    w1p[:IN] = w1
    w3r = w3.reshape(HID, HEADS, HID)

    attv = np.zeros((16, HID), np.float32)
    attv[0] = np.asarray(inputs["as1"], np.float32)[0]
    attv[1] = np.asarray(inputs["ad1"], np.float32)[0]
    attv[2] = w2 @ np.asarray(inputs["as2"], np.float32)[0]
    attv[3] = w2 @ np.asarray(inputs["ad2"], np.float32)[0]
    as3 = np.asarray(inputs["as3"], np.float32)
    ad3 = np.asarray(inputs["ad3"], np.float32)
    for h in range(HEADS):
        attv[4 + h] = w3r[:, h, :] @ as3[h]
        attv[7 + h] = w3r[:, h, :] @ ad3[h]

    # layer-1 attention folded into x @ (W1 @ [as1, ad1]^T)
    w1av = w1p @ attv[0:2].T          # [1152, 2]

    def fm_bias(b, parts):
        b = np.asarray(b, np.float32)
        return b.reshape(parts, 128).T.copy()

    # replicated per-layer row constants: [128, layer, {b,gamma,beta}, 1536]
    gbb = np.zeros((3, 3, HEADS * HID), np.float32)
    for l, nms in enumerate([("b1", "g1", "be1"), ("b2", "g2", "be2"),
                             ("b3", "g3", "be3")]):
        for j, nm in enumerate(nms):
            v = np.asarray(inputs[nm], np.float32)
            gbb[l, j, :v.shape[0]] = v
    gbbrep = np.broadcast_to(gbb, (128,) + gbb.shape).copy()
    gb_trivial = bool((gbb[:, 1] == 1.0).all() and (gbb[:, 2] == 0.0).all())
    # transposed attention vectors (edge-payload dots on PE, layers 2-3):
    # [layer, blk, 128, 6] with feature on the 128 axis
    att = np.zeros((2, 6, HID), np.float32)
    att[0, 0], att[0, 1] = attv[2], attv[3]
    att[1, 0:3], att[1, 3:6] = attv[4:7], attv[7:10]
    attT = np.ascontiguousarray(
        att.reshape(2, 6, 4, 128).transpose(0, 2, 3, 1)).astype(bf)

    shared = dict(
        w1p=w1p.astype(bf), w2=w2.astype(bf), w3=w3.astype(bf),
        w1av=np.ascontiguousarray(w1av.astype(bf)),
        gbbrep=gbbrep, attT=attT,
        rw=np.asarray(inputs["rw"], np.float32),
        l1w=np.asarray(inputs["l1w"], np.float32),
        l2w=np.asarray(inputs["l2w"], np.float32),
        ow=np.asarray(inputs["ow"], np.float32),
        rbp=fm_bias(inputs["rb"], 4), l1bp=fm_bias(inputs["l1b"], 2),
        l2bp=fm_bias(inputs["l2b"], 1), obp=fm_bias(inputs["ob"], 2),
        cinv=cinv,
    )

    in_maps = []
    for k in range(NCORES):
        m = dict(shared)
        m.update(per_core[k])
        in_maps.append({"blob": _pack_blob(m, KT)})
    return (KT, gb_trivial), in_maps


# ----------------------------------------------------------------------------
# device program
# ----------------------------------------------------------------------------

def build(key):
    KT, gb_trivial = key
    STAGE = int(os.environ.get('KSTAGE', '99'))
    KEDGE = int(os.environ.get('KEDGE', '0'))  # 1: no adg, 2: no msg, 3: neither
    ESL = KT * 128
    nc = bacc.Bacc("TRN2", target_bir_lowering=False, debug=False,
                   enable_asserts=False, num_devices=NCORES)

    LAY, TOTAL = _blob_layout(KT)
    i_blob = nc.dram_tensor("blob", [TOTAL // 4], dt.int32,
                            kind="ExternalInput")
    _DT = {"int16": dt.int16, "bf16": dt.bfloat16, "f32": dt.float32,
           "f32r": dt.float32r}

    def bap(name, pattern=None, **axes):
        """Shaped AP into the packed blob for tensor `name`."""
        off, shape, d = LAY[name]
        n = int(np.prod(shape))
        ap = i_blob[off // 4:(off + n * _DTSZ[d]) // 4].bitcast(_DT[d])
        if pattern is None:
            nms = [f"a{i}" for i in range(len(shape))]
            pattern = "(" + " ".join(nms) + ") -> " + " ".join(nms)
            axes = {f"a{i}": int(s) for i, s in enumerate(shape[:-1])}
        return ap.rearrange(pattern, **axes)

    i_sidx = bap("sidx")
    i_didx = bap("didx")
    i_st = bap("st")
    i_xt = bap("xt", "(c t k m) -> c k t m", c=CHUNKS, t=INP // 128, k=128)
    i_segm = bap("segm")
    i_segmT = bap("segmT")
    i_w1p = bap("w1p", "(t k m) -> k t m", t=INP // 128, k=128)
    i_w1av = bap("w1av", "(t k m) -> k t m", t=INP // 128, k=128)
    i_w2 = bap("w2", "(t k m) -> k t m", t=HID // 128, k=128)
    i_w3 = bap("w3", "(t k m) -> k t m", t=HID // 128, k=128)
    i_gbbrep = bap("gbbrep")
    i_attT = bap("attT", "(l t k m) -> l k t m", l=2, t=4, k=128)
    i_rw = bap("rw", "(t k m) -> k t m", t=HEADS * HID // 128, k=128)
    i_l1w = bap("l1w", "(t k m) -> k t m", t=HID // 128, k=128)
    i_l2w = bap("l2w", "(t k m) -> k t m", t=HID // 256, k=128)
    i_ow = bap("ow")
    i_rbp, i_l1bp = bap("rbp"), bap("l1bp")
    i_l2bp, i_obp = bap("l2bp"), bap("obp")
    i_cinv = bap("cinv")

    o_out = nc.dram_tensor("out", [B, OUT], dt.float32, kind="ExternalOutput")

    with tile.TileContext(nc) as tc:
        with (
            tc.tile_pool(name="const", bufs=1) as cp,
            tc.tile_pool(name="wbig", bufs=1) as wb,
            tc.tile_pool(name="stream", bufs=3) as sp,
            tc.tile_pool(name="msgp", bufs=2) as mp,
            tc.tile_pool(name="work", bufs=2) as wp,
            tc.tile_pool(name="psum", bufs=6, space="PSUM") as pp,
            tc.tile_pool(name="psb", bufs=2, space="PSUM") as psb,
            tc.tile_pool(name="dram", bufs=1, space="DRAM") as dr,
        ):
            f32, f32r, bf16 = dt.float32, dt.float32r, dt.bfloat16

            # ---- payload-critical loads first: everything else is
            # deferred until after the layer-1 payload issues, so the
            # SP DMA queue (and the Pool queue ahead of the first
            # AllGather) stays clear at startup ----
            w1t = wb.tile([128, INP // 128, HID], bf16, tag="wbig", name="w1t")
            nc.sync.dma_start(w1t[:], i_w1p)
            w1avt = cp.tile([128, INP // 128, 2], bf16)
            nc.sync.dma_start(w1avt[:], i_w1av)
            identf = cp.tile([128, 128], f32)
            make_identity(nc, identf[:])
            ident_b = cp.tile([128, 128], bf16)
            nc.vector.tensor_copy(ident_b[:], identf[:])
            identb = ident_b[:]
            ident_r = cp.tile([128, 128], f32r)
            nc.vector.tensor_copy(ident_r[:], identf[:])
            ident = ident_r[:]
            ones2 = cp.tile([128, 2], bf16)
            nc.vector.memset(ones2[:], 1.0)

            # persistent pre-norm activations (all 8 chunks stay in SBUF)
            o_all = cp.tile([128, CHUNKS, HEADS * HID], bf16, name="o_all")
            st_all = cp.tile([128, CHUNKS, KT, 128], bf16, name="st_all")
            sidx = cp.tile([128, CHUNKS, ESL // 16], dt.int16)
            didx = cp.tile([128, CHUNKS * ESL // 16], dt.int16)
            segm = cp.tile([128, CHUNKS, B], f32r)
            segmb = cp.tile([128, CHUNKS, B], bf16)
            segmT = cp.tile([B, CHUNKS, 128], f32r)
            w2t = cp.tile([128, HID // 128, HID], bf16, tag="wshare",
                          name="w2t")
            w3t = cp.tile([128, HID // 128, HEADS * HID], bf16)
            mlpb = {}
            for nm, t in [("rbp", i_rbp), ("l1bp", i_l1bp),
                          ("l2bp", i_l2bp), ("obp", i_obp)]:
                mlpb[nm] = cp.tile([128, t.shape[1]], f32, tag=f"mlpb_{nm}",
                                   name=f"mlpb_{nm}")
            cinv = cp.tile([B, 4], f32)

            def load_statics():
                # dst-indicator tiles, shared by all 3 layers
                nc.sync.dma_start(st_all[:],
                                  i_st.rearrange("c p k m -> p c k m"))
                nc.sync.dma_start(sidx[:], i_sidx)
                nc.sync.dma_start(didx[:], i_didx)
                nc.sync.dma_start(segm[:], i_segm)
                nc.vector.tensor_copy(segmb[:], segm[:].bitcast(f32))
                nc.sync.dma_start(segmT[:], i_segmT)
                nc.sync.dma_start(w2t[:], i_w2)
                nc.sync.dma_start(w3t[:], i_w3)
                for nm, t in [("rbp", i_rbp), ("l1bp", i_l1bp),
                              ("l2bp", i_l2bp), ("obp", i_obp)]:
                    nc.sync.dma_start(mlpb[nm][:], t)
                nc.sync.dma_start(cinv[:], i_cinv)

            # ---- DRAM scratch ----
            ag_in = [dr.tile([NSLOT, PW], bf16, tag=f"agin{l}", name=f"agin{l}")
                     for l in range(3)]
            ag_out = [dr.tile([GROWS, PW], bf16, tag=f"agout{l}",
                              name=f"agout{l}", addr_space="Shared")
                      for l in range(3)]
            ar_in = [dr.tile([B, 4], f32, tag=f"arin{l}", name=f"arin{l}")
                     for l in range(3)]
            ar_out = [dr.tile([B, 4], f32, tag=f"arout{l}", name=f"arout{l}",
                              addr_space="Shared") for l in range(3)]
            arp_in = dr.tile([B, HEADS * HID], f32, name="arp_in")
            arp_out = dr.tile([B, HEADS * HID], f32, name="arp_out",
                              addr_space="Shared")

            RG = [list(range(NCORES))]

            def load_gbias(lyr):
                # bias row, double-buffered so the next layer's prefetch
                # overlaps the edge phase
                gb = cp.tile([128, HEADS * HID], f32, tag=f"gbias{lyr % 2}",
                             name=f"gbias{lyr}")
                nc.sync.dma_start(gb[:], i_gbbrep[:, lyr, 0])
                return gb

            def load_ggb(lyr):
                # gamma/beta rows (bf16), single slot: only used in the norm
                # phase, so the next layer's load has a full edge phase of
                # slack
                if gb_trivial:
                    return None
                ggb = cp.tile([128, 2, HEADS * HID], bf16, tag="ggb",
                              name=f"ggb{lyr}")
                nc.gpsimd.dma_start(ggb[:], i_gbbrep[:, lyr, 1:3])
                return ggb

            def load_av(lyr):
                # transposed edge-payload attention vectors for layers 2
                # (lyr=1) and 3 (lyr=2); layer 1 folds into the x@W1 matmul.
                av = cp.tile([128, 4, 6], bf16, tag=f"av{lyr % 2}",
                             name=f"av{lyr}")
                nc.sync.dma_start(av[:], i_attT[lyr - 1])
                return av

            def attn_payload(lyr, c, act_tile, av):
                """Write payload row block c for gather-layer lyr and DMA it.

                act_tile: [128, HID] f32-ish SBUF tile (post norm+gelu).
                av: [128, 6, HID] replicated attention vectors."""
                H = HEADS if lyr == 2 else 1
                pay = sp.tile([128, PW], bf16, tag="pay", name="pay", bufs=3)
                nc.vector.tensor_copy(pay[:, :HID], act_tile[:, :HID])
                # attention dots on PE: transpose act, matmul with attT
                atp = pp.tile([128, 512], bf16, tag="big", name="atp")
                for i in range(4):
                    nc.tensor.transpose(atp[:, i * 128:(i + 1) * 128],
                                        act_tile[:, i * 128:(i + 1) * 128],
                                        identb)
                atsb = wp.tile([128, 4, 128], bf16, tag="tsb", name="atsb")
                nc.vector.tensor_copy(atsb[:], atp[:].rearrange(
                    "p (i k) -> p i k", i=4))
                att_ps = psb.tile([128, 2 * HEADS], f32, tag="sm", name="att_ps")
                for i in range(4):
                    nc.tensor.matmul(att_ps[:, :2 * H], atsb[:, i, :],
                                     av[:, i, :2 * H],
                                     start=(i == 0), stop=(i == 3))
                nc.vector.tensor_copy(pay[:, HID:HID + 2 * H],
                                      att_ps[:, :2 * H])
                nc.vector.memset(pay[:, HID + 2 * H:], 0.0)
                nc.sync.dma_start(ag_in[lyr][c * 128:(c + 1) * 128, :], pay[:])

            def allgather(lyr, half):
                if half == 0 or os.environ.get("KAG", "0") == "1":
                    return
                nc.gpsimd.collective_compute(
                    "AllGather", OP.bypass, replica_groups=RG,
                    ins=[ag_in[lyr].opt()], outs=[ag_out[lyr].opt()])

            def apply_w(msum, wtiles, wslice, out_ps, start, stop):
                """out_ps [128,512] (+)= msum [128,512] @ W via PE transpose.

                msum: [128, 512] bf16."""
                tps = pp.tile([128, 512], bf16, tag="big", name="tps")
                for i in range(4):
                    nc.tensor.transpose(tps[:, i * 128:(i + 1) * 128],
                                        msum[:, i * 128:(i + 1) * 128], identb)
                tsb = wp.tile([128, 4, 128], bf16, tag="tsb", name="tsb")
                nc.vector.tensor_copy(tsb[:], tps[:].rearrange("p (i k) -> p i k", i=4))
                for i in range(4):
                    nc.tensor.matmul(out_ps[:], tsb[:, i, :], wtiles[:, i, wslice],
                                     start=start and i == 0, stop=stop and i == 3)

            KBCAST = os.environ.get("KBCAST", "1") == "1"

            def edge_chunk(lyr, c, adg):
                """Edge phase for one chunk: per-head normalized msum tiles."""
                H = HEADS if lyr == 2 else 1
                msg = mp.tile([128, KT, PW], bf16, tag="msg", name="msg",
                              bufs=3)
                if KEDGE in (2, 3):
                    nc.vector.memset(msg[:], 0.0)
                else:
                    nc.gpsimd.dma_gather(
                        out_ap=msg[:], in_ap=ag_out[lyr][:],
                        idxs_ap=sidx[:, c, :], num_idxs=ESL, num_idxs_reg=ESL,
                        elem_size=PW, single_packet=False)
                stt = st_all[:, c]
                pte = wp.tile([128, KT, HEADS], f32, tag="pte", name="pte")
                nc.vector.tensor_tensor(
                    out=pte[:, :, :H], in0=msg[:, :, HID:HID + H],
                    in1=adg[:, c * KT:(c + 1) * KT, H:2 * H],
                    op=OP.add)
                nc.vector.scalar_tensor_tensor(
                    out=pte[:, :, :H], in0=pte[:, :, :H], scalar=SLOPE,
                    in1=pte[:, :, :H], op0=OP.mult, op1=OP.max)
                nc.scalar.activation(pte[:, :, :H], pte[:, :, :H], AF.Exp)
                def head_msum(h):
                    msum_ps = pp.tile([128, HID], f32, tag="big", name="msum_ps")
                    s_ps = psb.tile([128, 2], f32, tag="sm", name="s_ps")
                    Pall = None
                    if KBCAST:
                        Pall = wp.tile([128, KT, 128], bf16, tag="ptall",
                                       name="ptall")
                        nc.vector.tensor_tensor(
                            out=Pall[:], in0=stt,
                            in1=pte[:, :, h:h + 1].broadcast_to([128, KT, 128]),
                            op=OP.mult)
                    for b in range(KT):
                        if KBCAST:
                            P = Pall[:, b, :]
                        else:
                            Pt = wp.tile([128, 128], bf16, tag="ptile",
                                         name="ptile")
                            nc.vector.tensor_scalar_mul(
                                out=Pt[:], in0=stt[:, b],
                                scalar1=pte[:, b, h:h + 1])
                            P = Pt[:]
                        nc.tensor.matmul(msum_ps[:], P, msg[:, b, :HID],
                                         start=(b == 0), stop=(b == KT - 1))
                        nc.tensor.matmul(s_ps[:, :2], P, ones2[:],
                                         start=(b == 0), stop=(b == KT - 1))
                    rec = wp.tile([128, 1], f32, tag="rec", name="rec")
                    nc.vector.tensor_scalar_add(out=rec[:], in0=s_ps[:, 0:1],
                                                scalar1=EPS_SM)
                    nc.vector.reciprocal(rec[:], rec[:])
                    md = f32r if lyr == 0 else bf16
                    msum = wp.tile([128, HID], md, tag="msum", name="msum")
                    nc.vector.tensor_scalar_mul(out=msum[:], in0=msum_ps[:],
                                                scalar1=rec[:])
                    return msum
                return head_msum

            def layer_edge_to_out(lyr, gbias):
                """Edge phase + weight application; writes out rows to the
                SBUF-resident o_all and accumulates graph-norm partial sums;
                then AllReduces stats."""
                F = HEADS * HID if lyr == 2 else HID
                st1 = pp.tile([B, 512], f32, tag="big", name="st1")
                st2 = pp.tile([B, 512], f32, tag="big", name="st2")
                nslice = F // 512
                # one gather for all chunks' dst attention columns
                adg = mp.tile([128, CHUNKS * KT, AW], bf16, tag="adg",
                              name="adg", bufs=1)
                if KEDGE in (1, 3):
                    nc.vector.memset(adg[:], 0.0)
                else:
                    nc.gpsimd.dma_gather(
                        out_ap=adg[:], in_ap=ag_out[lyr][:, HID:],
                        idxs_ap=didx[:],
                        num_idxs=CHUNKS * ESL, num_idxs_reg=CHUNKS * ESL,
                        elem_size=AW, elem_step=PW,
                        single_packet=False)
                for c in range(CHUNKS):
                    head_msum = edge_chunk(lyr, c, adg)
                    H = HEADS if lyr == 2 else 1
                    for h in range(H):
                        msum = head_msum(h)
                        sl = slice(h * HID, (h + 1) * HID)
                        if lyr == 0:
                            nc.vector.tensor_tensor(
                                out=o_all[:, c, :HID], in0=msum[:].bitcast(f32),
                                in1=gbias[:, :HID], op=OP.add)
                        else:
                            wt = w2t if lyr == 1 else w3t
                            o_ps = pp.tile([128, HID], f32, tag="big",
                                           name="o_ps")
                            apply_w(msum, wt, sl, o_ps, True, True)
                            nc.vector.tensor_tensor(
                                out=o_all[:, c, sl], in0=o_ps[:],
                                in1=gbias[:, sl], op=OP.add)
                    for s in range(nslice):
                        sl = slice(s * 512, (s + 1) * 512)
                        sq = wp.tile([128, 512], bf16, tag="scratch", name="sq")
                        nc.vector.tensor_tensor(out=sq[:], in0=o_all[:, c, sl],
                                                in1=o_all[:, c, sl], op=OP.mult)
                        first = c == 0 and s == 0
                        last = c == CHUNKS - 1 and s == nslice - 1
                        nc.tensor.matmul(st1[:, :512], segmb[:, c, :],
                                         o_all[:, c, sl],
                                         start=first, stop=last)
                        nc.tensor.matmul(st2[:, :512], segmb[:, c, :], sq[:],
                                         start=first, stop=last)
                s12 = wp.tile([B, 4], f32, tag="s12", name="s12")
                nc.vector.memset(s12[:], 0.0)
                nc.vector.reduce_sum(s12[:, 0:1], st1[:], axis=mybir.AxisListType.X)
                nc.vector.reduce_sum(s12[:, 1:2], st2[:], axis=mybir.AxisListType.X)
                nc.sync.dma_start(ar_in[lyr][:], s12[:])
                if os.environ.get("KAR", "0") == "1":
                    nc.sync.dma_start(ar_out[lyr][:], s12[:])
                    return
                nc.gpsimd.collective_compute(
                    "AllReduce", OP.add, replica_groups=RG,
                    ins=[ar_in[lyr].opt()], outs=[ar_out[lyr].opt()])

            def layer_norm_consume(lyr, ggb, consume):
                """Normalize + gelu the SBUF-resident rows, hand tiles on."""
                F = HEADS * HID if lyr == 2 else HID
                ccol = 1 if lyr == 2 else 0
                sg = wp.tile([B, 4], f32, tag="s12", name="sg")
                nc.sync.dma_start(sg[:], ar_out[lyr][:])
                mu = wp.tile([B, 4], f32, tag="mu", name="mu")
                nc.vector.tensor_tensor(out=mu[:, 0:1], in0=sg[:, 0:1],
                                        in1=cinv[:, ccol:ccol + 1], op=OP.mult)
                nc.vector.tensor_tensor(out=mu[:, 2:3], in0=sg[:, 1:2],
                                        in1=cinv[:, ccol:ccol + 1], op=OP.mult)
                nc.vector.tensor_tensor(out=mu[:, 3:4], in0=mu[:, 0:1],
                                        in1=mu[:, 0:1], op=OP.mult)
                nc.vector.tensor_tensor(out=mu[:, 2:3], in0=mu[:, 2:3],
                                        in1=mu[:, 3:4], op=OP.subtract)
                nc.vector.tensor_scalar_add(out=mu[:, 2:3], in0=mu[:, 2:3],
                                            scalar1=EPS_LN)
                nc.scalar.activation(mu[:, 2:3], mu[:, 2:3], AF.Sqrt)
                nc.vector.reciprocal(mu[:, 1:2], mu[:, 2:3])
                stats2 = wp.tile([B, 2], f32r, tag="stats2", name="stats2")
                nc.vector.tensor_copy(stats2[:], mu[:, 0:2])
                # burst-precompute per-node (mean, 1/std) for all chunks so
                # the norm loop below never waits on tiny PE matmuls
                nst = wp.tile([128, CHUNKS, 2], f32, tag="nst", name="nst")
                for c in range(CHUNKS):
                    nst_ps = psb.tile([128, 2], f32, tag="sm", name="nst_ps")
                    nc.tensor.matmul(nst_ps[:], segmT[:, c, :], stats2[:],
                                     start=True, stop=True)
                    nc.vector.tensor_copy(nst[:, c, :], nst_ps[:])
                for c in range(CHUNKS):
                    act = wp.tile([128, HEADS * HID], bf16, tag="actc",
                                  name="actc", bufs=3)
                    nc.vector.tensor_scalar(
                        out=act[:, :F], in0=o_all[:, c, :F],
                        scalar1=nst[:, c, 0:1], scalar2=nst[:, c, 1:2],
                        op0=OP.subtract, op1=OP.mult)
                    if not gb_trivial:
                        nc.vector.tensor_tensor(
                            out=act[:, :F], in0=act[:, :F],
                            in1=ggb[:, 0, :F], op=OP.mult)
                        nc.vector.tensor_tensor(
                            out=act[:, :F], in0=act[:, :F],
                            in1=ggb[:, 1, :F], op=OP.add)
                    nc.scalar.activation(act[:, :F], act[:, :F], AF.Gelu)
                    consume(c, act)

            # ================= layer 1 payload =================
            for c in range(CHUNKS):
                h1_ps = pp.tile([128, HID], f32, tag="big", name="h1_ps")
                av_ps = psb.tile([128, 2], f32, tag="sm", name="av_ps")
                xt_t = sp.tile([128, INP // 128, 128], bf16, tag="xt", name="xt")
                nc.sync.dma_start(xt_t[:], i_xt[c])
                for t in range(INP // 128):
                    nc.tensor.matmul(h1_ps[:], xt_t[:, t, :], w1t[:, t, :],
                                     start=(t == 0), stop=(t == INP // 128 - 1))
                    nc.tensor.matmul(av_ps[:], xt_t[:, t, :], w1avt[:, t, :],
                                     start=(t == 0), stop=(t == INP // 128 - 1))
                pay = sp.tile([128, PW], bf16, tag="pay", name="pay", bufs=3)
                nc.vector.tensor_copy(pay[:, :HID], h1_ps[:])
                nc.vector.tensor_copy(pay[:, HID:HID + 2], av_ps[:])
                nc.vector.memset(pay[:, HID + 2:], 0.0)
                nc.sync.dma_start(ag_in[0][c * 128:(c + 1) * 128, :], pay[:])
                if c == 3:
                    allgather(0, 0)
            allgather(0, 1)
            load_statics()

            # ================= layers =================
            if STAGE >= 1:
                gb1 = load_gbias(0)
                ggb1 = load_ggb(0)
                av1 = load_av(1)
                layer_edge_to_out(0, gb1)
            if STAGE >= 2:
                gb2 = load_gbias(1)
                def consume1(c, act):
                    attn_payload(1, c, act, av1)
                    if c == 3:
                        allgather(1, 0)
                layer_norm_consume(0, ggb1, consume1)
                allgather(1, 1)
            if STAGE >= 3:
                ggb2 = load_ggb(1)
                av2 = load_av(2)
                layer_edge_to_out(1, gb2)
                gb3 = load_gbias(2)
                def consume2(c, act):
                    attn_payload(2, c, act, av2)
                    if c == 3:
                        allgather(2, 0)
                layer_norm_consume(1, ggb2, consume2)
                allgather(2, 1)
            if STAGE >= 4:
                ggb3 = load_ggb(2)
                layer_edge_to_out(2, gb3)
                pool_ps = [pp.tile([B, 512], f32, tag="big", name=f"pool{s}")
                           for s in range(3)]

                def pool_consume(c, act):
                    for s in range(3):
                        nc.tensor.matmul(pool_ps[s][:], segmb[:, c, :],
                                         act[:, s * 512:(s + 1) * 512],
                                         start=(c == 0), stop=(c == CHUNKS - 1))

                layer_norm_consume(2, ggb3, pool_consume)

            if STAGE < 4:
                fin0 = wp.tile([B, OUT], f32, tag="scratch", name="fin0")
                nc.vector.memset(fin0[:], 0.0)
                nc.sync.dma_start(o_out[:], fin0[:])
            # ================= pooling + MLP =================
            if STAGE >= 4:
                pool_sb = wp.tile([B, HEADS * HID], f32, tag="outc", name="poolsb")
                for s in range(3):
                    nc.vector.tensor_copy(pool_sb[:, s * 512:(s + 1) * 512],
                                          pool_ps[s][:])
                nc.sync.dma_start(arp_in[:], pool_sb[:])
                nc.gpsimd.collective_compute(
                    "AllReduce", OP.add, replica_groups=RG,
                    ins=[arp_in.opt()], outs=[arp_out.opt()])
                pooled = wp.tile([B, HEADS * HID], f32r, tag="outc", name="pooled")
                nc.sync.dma_start(pooled[:], arp_out[:].bitcast(f32r))
                nc.vector.tensor_scalar_mul(out=pooled[:], in0=pooled[:].bitcast(f32),
                                            scalar1=cinv[:, 2:3])

                # MLP weights reuse the w1t slot
                rwt = wb.tile([128, 12, HID], bf16, tag="wbig", name="rwt")
                nc.gpsimd.dma_start(rwt[:], i_rw)
                l1wt = cp.tile([128, 4, HID // 2], bf16, tag="wshare", name="l1wt")
                nc.gpsimd.dma_start(l1wt[:], i_l1w)
                l2wt = cp.tile([128, 2, HID // 4], bf16)
                nc.gpsimd.dma_start(l2wt[:], i_l2w)
                owt = cp.tile([128, OUT], bf16)
                nc.gpsimd.dma_start(owt[:], i_ow)

                pT = wp.tile([128, 12, B], bf16, tag="msum", name="pT")
                for i in range(12):
                    ps = psb.tile([128, B], f32r, tag="sm", name="pT_ps")
                    nc.tensor.transpose(ps[:], pooled[:, i * 128:(i + 1) * 128],
                                        ident[:B, :B])
                    nc.vector.tensor_copy(pT[:, i, :], ps[:])

                def mlp_layer(in_t, nkt, wtiles, nm, brow, actf, nm_tag):
                    outt = wp.tile([128, nm, B], bf16, tag=nm_tag, name=nm_tag)
                    for m in range(nm):
                        ps = psb.tile([128, B], f32, tag="sm", name="mlp_ps")
                        for t in range(nkt):
                            nc.tensor.matmul(ps[:], wtiles[:, t, m * 128:(m + 1) * 128],
                                             in_t[:, t, :],
                                             start=(t == 0), stop=(t == nkt - 1))
                        nc.scalar.activation(outt[:, m, :], ps[:], actf,
                                             bias=brow[:, m:m + 1])
                    return outt

                m1 = mlp_layer(pT, 12, rwt, 4, mlpb["rbp"], AF.Gelu, "mlp1")
                m2 = mlp_layer(m1, 4, l1wt, 2, mlpb["l1bp"], AF.Gelu, "mlp2")
                m3 = mlp_layer(m2, 2, l2wt, 1, mlpb["l2bp"], AF.Gelu, "mlp3")
                mf = wp.tile([128, 2, B], bf16, tag="mlp1", name="mf")
                for m in range(2):
                    ps = psb.tile([128, B], f32, tag="sm", name="mf_ps")
                    nc.tensor.matmul(ps[:], owt[:, m * 128:(m + 1) * 128], m3[:, 0, :],
                                     start=True, stop=True)
                    nc.scalar.activation(mf[:, m, :], ps[:], AF.Identity,
                                         bias=mlpb["obp"][:, m:m + 1])
                fin = wp.tile([B, OUT], f32, tag="finout", name="fin")
                for m in range(2):
                    ps = psb.tile([B, 128], bf16, tag="sm", name="fin_ps")
                    nc.tensor.transpose(ps[:], mf[:, m, :], identb)
                    nc.vector.tensor_copy(fin[:, m * 128:(m + 1) * 128], ps[:])
                nc.sync.dma_start(o_out[:], fin[:])

    nc.compile()
    return nc


_CACHE = {}


def kernel(**inputs):
    key, in_maps = preprocess(inputs)
    if key not in _CACHE:
        _CACHE[key] = build(key)
    nc = _CACHE[key]
    res = run_bass_kernel_spmd(nc, in_maps, core_ids=list(range(NCORES)))
    return res.results[0]["out"].astype(np.float32)


# revision 61
# speedup vs baseline: 1.1086x; 1.1086x over previous
"""3-layer GAT (nn_GATfly3) on 8 TRN2 NeuronCores.

Strategy (v3)
-------------
Nodes are sharded across the 8 cores (1000 each, padded to 1024 slots =
8 chunks x 128); each core owns its nodes' incident edges (by dst).
All inputs ship as ONE packed int32 blob per core (device-side APs
bitcast/reshape slices of it) to minimize per-dispatch marshaling.
Per layer:
  1. every core computes payload rows [feat(512)|a_s|a_d|pad] in BF16
     for its nodes and all-gathers them into a Shared DRAM table
     [8192 x 640] (row = 1280B, dma_gather needs %256B),
  2. edge phase: one dma_gather per layer pulls every edge's dst
     attention columns (256B slices); per 128-dst chunk a dma_gather
     pulls the src rows (bf16, 1280B each); softmax weights
     p = exp(leaky_relu(a_s+a_d)) are computed per edge (no per-segment
     max: |e| stays small enough for fp32 exp; the reference's
     max-subtraction cancels out of alpha up to the 1e-16 epsilon) and
     broadcast-multiplied into static 0/1 dst-indicator tiles S_T;
     a single PSUM accumulation per chunk then yields both the weighted
     message sum and the softmax denominator,
  3. the layer weight is applied AFTER the segment sum (linearity:
     segsum(alpha*(x@W)[src]) == segsum(alpha*x[src]) @ W), so gathers
     stay 512-wide even for the 3-head layer 3; matmuls/transposes run
     in bf16,
  4. pre-norm activations stay resident in SBUF (bf16) across the tiny
     graph-norm stats AllReduce; gamma/beta application is skipped when
     they are identity (rebuilds if not); pooling is one indicator
     matmul + AllReduce; the tiny MLP runs replicated on every core.
Layer-1 attention dots fold into the x@W1 matmul (x @ (W1@attv));
layer-2/3 payload dots run on the PE (transpose + matmul with attT).
"""

import math
import os

import numpy as np

import concourse.mybir as mybir
import concourse.tile as tile
from concourse import bacc
from concourse.bass_utils import run_bass_kernel_spmd
from concourse.masks import make_identity

dt = mybir.dt
AF = mybir.ActivationFunctionType
OP = mybir.AluOpType

# problem constants
N, E, B = 8000, 80000, 16
IN, HID, HEADS, OUT = 1025, 512, 3, 256
EPS_LN, EPS_SM, SLOPE = 1e-5, 1e-16, 0.2

NCORES = 8
NLOC = N // NCORES          # 1000 owned nodes per core
CHUNKS = 8                  # dst chunks per core
NSLOT = CHUNKS * 128        # 1024 node slots per core
GROWS = NCORES * NSLOT      # 8192 rows in the gathered payload table
INP = 1152                  # IN padded to 9*128
PW = 640                    # payload row width (bf16 -> 1280B, %256B)
AW = 128                    # attention column block (256B in bf16)


# ----------------------------------------------------------------------------
# host-side preprocessing
# ----------------------------------------------------------------------------

_DTSZ = {"int16": 2, "bf16": 2, "f32": 4, "f32r": 4}


def _blob_layout(KT):
    """Single packed input blob: name -> (byte offset, shape, dtype tag).
    512B-aligned entries; identical on host (packing) and device (APs)."""
    ESL = KT * 128
    specs = [
        ("sidx", (128, CHUNKS, ESL // 16), "int16"),
        ("didx", (128, CHUNKS * ESL // 16), "int16"),
        ("st", (CHUNKS, 128, KT, 128), "bf16"),
        ("xt", (CHUNKS, INP // 128, 128, 128), "bf16"),
        ("segm", (128, CHUNKS, B), "f32r"),
        ("segmT", (B, CHUNKS, 128), "f32r"),
        ("w1p", (INP, HID), "bf16"),
        ("w1av", (INP, 2), "bf16"),
        ("w2", (HID, HID), "bf16"),
        ("w3", (HID, HEADS * HID), "bf16"),
        ("gbbrep", (128, 3, 3, HEADS * HID), "f32"),
        ("attT", (2, 4, 128, 6), "bf16"),
        ("rw", (HEADS * HID, HID), "f32"),
        ("l1w", (HID, HID // 2), "f32"),
        ("l2w", (HID // 2, HID // 4), "f32"),
        ("ow", (HID // 4, OUT), "f32"),
        ("rbp", (128, 4), "f32"),
        ("l1bp", (128, 2), "f32"),
        ("l2bp", (128, 1), "f32"),
        ("obp", (128, 2), "f32"),
        ("cinv", (B, 4), "f32"),
    ]
    lay = {}
    off = 0
    for nm, shape, d in specs:
        nbytes = int(np.prod(shape)) * _DTSZ[d]
        lay[nm] = (off, shape, d)
        off += (nbytes + 511) // 512 * 512
    return lay, off


def _pack_blob(m, KT):
    lay, total = _blob_layout(KT)
    u8 = np.zeros(total, np.uint8)
    for nm, (off, shape, d) in lay.items():
        arr = np.ascontiguousarray(m[nm])
        b = arr.tobytes()
        assert len(b) == int(np.prod(shape)) * _DTSZ[d], \
            f"{nm}: {len(b)} != {shape} x {d}"
        u8[off:off + len(b)] = np.frombuffer(b, np.uint8)
    return u8.view(np.int32)


def _wrap_idx(idx, nslots):
    """Edge-slot indices -> dma_gather SBUF layout [128, nslots//16].

    Index i is read from partition i%16, column i//16; the 16-row block is
    replicated across the 8 GPSIMD core groups (partitions 0..127)."""
    assert len(idx) == nslots and nslots % 16 == 0
    w = np.zeros((16, nslots // 16), np.int16)
    w[np.arange(nslots) % 16, np.arange(nslots) // 16] = idx.astype(np.int16)
    return np.tile(w, (8, 1))


def preprocess(inputs):
    import ml_dtypes
    bf = ml_dtypes.bfloat16
    x = np.asarray(inputs["x"], np.float32)
    ei = np.asarray(inputs["edge_index"], np.int64)
    batch = np.asarray(inputs["batch"], np.int64)
    src_g, dst_g = ei[0], ei[1]

    # ---- per-core node -> slot assignment (edge-balanced chunks) ----
    deg = np.bincount(dst_g, minlength=N)
    slot_of_node = np.full(N, -1, np.int64)
    node_of_slot = [np.full(NSLOT, -1, np.int64) for _ in range(NCORES)]
    for k in range(NCORES):
        nodes = np.arange(k * NLOC, (k + 1) * NLOC)
        order = nodes[np.argsort(-deg[nodes], kind="stable")]
        load = np.zeros(CHUNKS, np.int64)
        fill = np.zeros(CHUNKS, np.int64)
        for n in order:
            cands = np.where(fill < 128)[0]
            c = cands[np.argmin(load[cands])]
            slot_of_node[n] = c * 128 + fill[c]
            node_of_slot[k][c * 128 + fill[c]] = n
            fill[c] += 1
            load[c] += deg[n]
    core_of_node = np.arange(N) // NLOC
    grow_of_node = core_of_node * NSLOT + slot_of_node

    # ---- per-core edge lists grouped by (chunk, dst slot) ----
    KT = 0
    per_core_edges = []
    for k in range(NCORES):
        mask = core_of_node[dst_g] == k
        es, ed = src_g[mask], dst_g[mask]
        dslot = slot_of_node[ed]
        order = np.argsort(dslot, kind="stable")
        es, dslot = es[order], dslot[order]
        chunk = dslot // 128
        counts = np.bincount(chunk, minlength=CHUNKS)
        KT = max(KT, int(math.ceil(counts.max() / 128)))
        per_core_edges.append((es, dslot, chunk, counts))

    ESL = KT * 128  # edge slots per chunk

    cnt = np.bincount(batch, minlength=B).astype(np.float64)
    cntc = np.maximum(cnt, 1.0)
    cinv = np.zeros((B, 4), np.float32)
    cinv[:, 0] = 1.0 / (cntc * HID)
    cinv[:, 1] = 1.0 / (cntc * (HEADS * HID))
    cinv[:, 2] = 1.0 / cntc

    per_core = []
    for k in range(NCORES):
        es, dslot, chunk, counts = per_core_edges[k]
        st = np.zeros((CHUNKS, KT, 128, 128), np.float32)
        src_rows = np.zeros((CHUNKS, ESL), np.int64)
        dst_rows = np.zeros(CHUNKS * ESL, np.int64)
        for c in range(CHUNKS):
            sel = chunk == c
            ec = int(counts[c])
            s_c, d_c = es[sel], dslot[sel] - c * 128
            j = np.arange(ec)
            st[c, j // 128, j % 128, d_c] = 1.0
            src_rows[c, :ec] = grow_of_node[s_c]
            dst_rows[c * ESL: c * ESL + ec] = k * NSLOT + c * 128 + d_c
        sidx = np.stack([_wrap_idx(src_rows[c], ESL) for c in range(CHUNKS)], axis=1)
        didx = _wrap_idx(dst_rows, CHUNKS * ESL)
        # device layout [c, partition(edge), ktile, dst], bf16 (0/1 exact)
        st = np.ascontiguousarray(np.transpose(st, (0, 2, 1, 3)))
        st = st.astype(bf)

        xt = np.zeros((CHUNKS, INP // 128, 128, 128), np.float32)
        xpad = np.zeros((NSLOT, INP), np.float32)
        valid = node_of_slot[k] >= 0
        xpad[valid, :IN] = x[node_of_slot[k][valid]]
        xr = xpad.reshape(CHUNKS, 128, INP // 128, 128)
        xt[:] = np.transpose(xr, (0, 2, 3, 1))

        segm = np.zeros((CHUNKS, 128, B), np.float32)
        segmT = np.zeros((CHUNKS, B, 128), np.float32)
        bslot = np.full(NSLOT, -1, np.int64)
        bslot[valid] = batch[node_of_slot[k][valid]]
        for c in range(CHUNKS):
            for p in range(128):
                g = bslot[c * 128 + p]
                if g >= 0:
                    segm[c, p, g] = 1.0
                    segmT[c, g, p] = 1.0

        per_core.append(dict(
            sidx=sidx, didx=didx, st=st, xt=xt.astype(bf),
            segm=np.ascontiguousarray(np.transpose(segm, (1, 0, 2))),
            segmT=np.ascontiguousarray(np.transpose(segmT, (1, 0, 2)))))

    # ---- shared (replicated) weights ----
    w1 = np.asarray(inputs["w1"], np.float32)
    w2 = np.asarray(inputs["w2"], np.float32)
    w3 = np.asarray(inputs["w3"], np.float32)
    w1p = np.zeros((INP, HID), np.float32)
    w1p[:IN] = w1
    w3r = w3.reshape(HID, HEADS, HID)

    attv = np.zeros((16, HID), np.float32)
    attv[0] = np.asarray(inputs["as1"], np.float32)[0]
    attv[1] = np.asarray(inputs["ad1"], np.float32)[0]
    attv[2] = w2 @ np.asarray(inputs["as2"], np.float32)[0]
    attv[3] = w2 @ np.asarray(inputs["ad2"], np.float32)[0]
    as3 = np.asarray(inputs["as3"], np.float32)
    ad3 = np.asarray(inputs["ad3"], np.float32)
    for h in range(HEADS):
        attv[4 + h] = w3r[:, h, :] @ as3[h]
        attv[7 + h] = w3r[:, h, :] @ ad3[h]

    # layer-1 attention folded into x @ (W1 @ [as1, ad1]^T)
    w1av = w1p @ attv[0:2].T          # [1152, 2]

    def fm_bias(b, parts):
        b = np.asarray(b, np.float32)
        return b.reshape(parts, 128).T.copy()

    # replicated per-layer row constants: [128, layer, {b,gamma,beta}, 1536]
    gbb = np.zeros((3, 3, HEADS * HID), np.float32)
    for l, nms in enumerate([("b1", "g1", "be1"), ("b2", "g2", "be2"),
                             ("b3", "g3", "be3")]):
        for j, nm in enumerate(nms):
            v = np.asarray(inputs[nm], np.float32)
            gbb[l, j, :v.shape[0]] = v
    gbbrep = np.broadcast_to(gbb, (128,) + gbb.shape).copy()
    gb_trivial = bool((gbb[:, 1] == 1.0).all() and (gbb[:, 2] == 0.0).all())
    # transposed attention vectors (edge-payload dots on PE, layers 2-3):
    # [layer, blk, 128, 6] with feature on the 128 axis
    att = np.zeros((2, 6, HID), np.float32)
    att[0, 0], att[0, 1] = attv[2], attv[3]
    att[1, 0:3], att[1, 3:6] = attv[4:7], attv[7:10]
    attT = np.ascontiguousarray(
        att.reshape(2, 6, 4, 128).transpose(0, 2, 3, 1)).astype(bf)

    shared = dict(
        w1p=w1p.astype(bf), w2=w2.astype(bf), w3=w3.astype(bf),
        w1av=np.ascontiguousarray(w1av.astype(bf)),
        gbbrep=gbbrep, attT=attT,
        rw=np.asarray(inputs["rw"], np.float32),
        l1w=np.asarray(inputs["l1w"], np.float32),
        l2w=np.asarray(inputs["l2w"], np.float32),
        ow=np.asarray(inputs["ow"], np.float32),
        rbp=fm_bias(inputs["rb"], 4), l1bp=fm_bias(inputs["l1b"], 2),
        l2bp=fm_bias(inputs["l2b"], 1), obp=fm_bias(inputs["ob"], 2),
        cinv=cinv,
    )

    in_maps = []
    for k in range(NCORES):
        m = dict(shared)
        m.update(per_core[k])
        in_maps.append({"blob": _pack_blob(m, KT)})
    return (KT, gb_trivial), in_maps


# ----------------------------------------------------------------------------
# device program
# ----------------------------------------------------------------------------

def build(key):
    KT, gb_trivial = key
    STAGE = int(os.environ.get('KSTAGE', '99'))
    KEDGE = int(os.environ.get('KEDGE', '0'))  # 1: no adg, 2: no msg, 3: neither
    ESL = KT * 128
    nc = bacc.Bacc("TRN2", target_bir_lowering=False, debug=False,
                   enable_asserts=False, num_devices=NCORES)

    LAY, TOTAL = _blob_layout(KT)
    i_blob = nc.dram_tensor("blob", [TOTAL // 4], dt.int32,
                            kind="ExternalInput")
    _DT = {"int16": dt.int16, "bf16": dt.bfloat16, "f32": dt.float32,
           "f32r": dt.float32r}

    def bap(name, pattern=None, **axes):
        """Shaped AP into the packed blob for tensor `name`."""
        off, shape, d = LAY[name]
        n = int(np.prod(shape))
        ap = i_blob[off // 4:(off + n * _DTSZ[d]) // 4].bitcast(_DT[d])
        if pattern is None:
            nms = [f"a{i}" for i in range(len(shape))]
            pattern = "(" + " ".join(nms) + ") -> " + " ".join(nms)
            axes = {f"a{i}": int(s) for i, s in enumerate(shape[:-1])}
        return ap.rearrange(pattern, **axes)

    i_sidx = bap("sidx")
    i_didx = bap("didx")
    i_st = bap("st")
    i_xt = bap("xt", "(c t k m) -> c k t m", c=CHUNKS, t=INP // 128, k=128)
    i_segm = bap("segm")
    i_segmT = bap("segmT")
    i_w1p = bap("w1p", "(t k m) -> k t m", t=INP // 128, k=128)
    i_w1av = bap("w1av", "(t k m) -> k t m", t=INP // 128, k=128)
    i_w2 = bap("w2", "(t k m) -> k t m", t=HID // 128, k=128)
    i_w3 = bap("w3", "(t k m) -> k t m", t=HID // 128, k=128)
    i_gbbrep = bap("gbbrep")
    i_attT = bap("attT", "(l t k m) -> l k t m", l=2, t=4, k=128)
    i_rw = bap("rw", "(t k m) -> k t m", t=HEADS * HID // 128, k=128)
    i_l1w = bap("l1w", "(t k m) -> k t m", t=HID // 128, k=128)
    i_l2w = bap("l2w", "(t k m) -> k t m", t=HID // 256, k=128)
    i_ow = bap("ow")
    i_rbp, i_l1bp = bap("rbp"), bap("l1bp")
    i_l2bp, i_obp = bap("l2bp"), bap("obp")
    i_cinv = bap("cinv")

    o_out = nc.dram_tensor("out", [B, OUT], dt.float32, kind="ExternalOutput")

    with tile.TileContext(nc) as tc:
        with (
            tc.tile_pool(name="const", bufs=1) as cp,
            tc.tile_pool(name="wbig", bufs=1) as wb,
            tc.tile_pool(name="stream", bufs=3) as sp,
            tc.tile_pool(name="msgp", bufs=2) as mp,
            tc.tile_pool(name="work", bufs=2) as wp,
            tc.tile_pool(name="psum", bufs=6, space="PSUM") as pp,
            tc.tile_pool(name="psb", bufs=2, space="PSUM") as psb,
            tc.tile_pool(name="dram", bufs=1, space="DRAM") as dr,
        ):
            f32, f32r, bf16 = dt.float32, dt.float32r, dt.bfloat16

            # ---- payload-critical loads first: everything else is
            # deferred until after the layer-1 payload issues, so the
            # SP DMA queue (and the Pool queue ahead of the first
            # AllGather) stays clear at startup ----
            w1t = wb.tile([128, INP // 128, HID], bf16, tag="wbig", name="w1t")
            nc.sync.dma_start(w1t[:], i_w1p)
            w1avt = cp.tile([128, INP // 128, 2], bf16)
            nc.sync.dma_start(w1avt[:], i_w1av)
            identf = cp.tile([128, 128], f32)
            make_identity(nc, identf[:])
            ident_b = cp.tile([128, 128], bf16)
            nc.vector.tensor_copy(ident_b[:], identf[:])
            identb = ident_b[:]
            ident_r = cp.tile([128, 128], f32r)
            nc.vector.tensor_copy(ident_r[:], identf[:])
            ident = ident_r[:]
            ones2 = cp.tile([128, 2], bf16)
            nc.vector.memset(ones2[:], 1.0)

            # persistent pre-norm activations (all 8 chunks stay in SBUF)
            o_all = cp.tile([128, CHUNKS, HEADS * HID], bf16, name="o_all")
            st_all = cp.tile([128, CHUNKS, KT, 128], bf16, name="st_all")
            sidx = cp.tile([128, CHUNKS, ESL // 16], dt.int16)
            didx = cp.tile([128, CHUNKS * ESL // 16], dt.int16)
            segm = cp.tile([128, CHUNKS, B], f32r)
            segmb = cp.tile([128, CHUNKS, B], bf16)
            segmT = cp.tile([B, CHUNKS, 128], f32r)
            w2t = cp.tile([128, HID // 128, HID], bf16, tag="wshare",
                          name="w2t")
            w3t = cp.tile([128, HID // 128, HEADS * HID], bf16)
            mlpb = {}
            for nm, t in [("rbp", i_rbp), ("l1bp", i_l1bp),
                          ("l2bp", i_l2bp), ("obp", i_obp)]:
                mlpb[nm] = cp.tile([128, t.shape[1]], f32, tag=f"mlpb_{nm}",
                                   name=f"mlpb_{nm}")
            cinv = cp.tile([B, 4], f32)

            def load_statics():
                # dst-indicator tiles, shared by all 3 layers
                nc.sync.dma_start(st_all[:],
                                  i_st.rearrange("c p k m -> p c k m"))
                nc.sync.dma_start(sidx[:], i_sidx)
                nc.sync.dma_start(didx[:], i_didx)
                nc.sync.dma_start(segm[:], i_segm)
                nc.vector.tensor_copy(segmb[:], segm[:].bitcast(f32))
                nc.sync.dma_start(segmT[:], i_segmT)
                nc.sync.dma_start(w2t[:], i_w2)
                nc.sync.dma_start(w3t[:], i_w3)
                for nm, t in [("rbp", i_rbp), ("l1bp", i_l1bp),
                              ("l2bp", i_l2bp), ("obp", i_obp)]:
                    nc.sync.dma_start(mlpb[nm][:], t)
                nc.sync.dma_start(cinv[:], i_cinv)

            # ---- DRAM scratch ----
            ag_in = [dr.tile([NSLOT, PW], bf16, tag=f"agin{l}", name=f"agin{l}")
                     for l in range(3)]
            ag_out = [dr.tile([GROWS, PW], bf16, tag=f"agout{l}",
                              name=f"agout{l}", addr_space="Shared")
                      for l in range(3)]
            ar_in = [dr.tile([B, 4], f32, tag=f"arin{l}", name=f"arin{l}")
                     for l in range(3)]
            ar_out = [dr.tile([B, 4], f32, tag=f"arout{l}", name=f"arout{l}",
                              addr_space="Shared") for l in range(3)]
            arp_in = dr.tile([B, HEADS * HID], f32, name="arp_in")
            arp_out = dr.tile([B, HEADS * HID], f32, name="arp_out",
                              addr_space="Shared")

            RG = [list(range(NCORES))]

            def load_gbias(lyr):
                # bias row, double-buffered so the next layer's prefetch
                # overlaps the edge phase
                gb = cp.tile([128, HEADS * HID], f32, tag=f"gbias{lyr % 2}",
                             name=f"gbias{lyr}")
                nc.sync.dma_start(gb[:], i_gbbrep[:, lyr, 0])
                return gb

            def load_ggb(lyr):
                # gamma/beta rows (bf16), single slot: only used in the norm
                # phase, so the next layer's load has a full edge phase of
                # slack
                if gb_trivial:
                    return None
                ggb = cp.tile([128, 2, HEADS * HID], bf16, tag="ggb",
                              name=f"ggb{lyr}")
                nc.gpsimd.dma_start(ggb[:], i_gbbrep[:, lyr, 1:3])
                return ggb

            def load_av(lyr):
                # transposed edge-payload attention vectors for layers 2
                # (lyr=1) and 3 (lyr=2); layer 1 folds into the x@W1 matmul.
                av = cp.tile([128, 4, 6], bf16, tag=f"av{lyr % 2}",
                             name=f"av{lyr}")
                nc.sync.dma_start(av[:], i_attT[lyr - 1])
                return av

            def attn_payload(lyr, c, act_tile, av):
                """Write payload row block c for gather-layer lyr and DMA it.

                act_tile: [128, HID] f32-ish SBUF tile (post norm+gelu).
                av: [128, 6, HID] replicated attention vectors."""
                H = HEADS if lyr == 2 else 1
                pay = sp.tile([128, PW], bf16, tag="pay", name="pay", bufs=2)
                nc.vector.tensor_copy(pay[:, :HID], act_tile[:, :HID])
                # attention dots on PE: transpose act, matmul with attT
                atp = pp.tile([128, 512], bf16, tag="big", name="atp")
                for i in range(4):
                    nc.tensor.transpose(atp[:, i * 128:(i + 1) * 128],
                                        act_tile[:, i * 128:(i + 1) * 128],
                                        identb)
                atsb = wp.tile([128, 4, 128], bf16, tag="tsb", name="atsb")
                nc.vector.tensor_copy(atsb[:], atp[:].rearrange(
                    "p (i k) -> p i k", i=4))
                att_ps = psb.tile([128, 2 * HEADS], f32, tag="sm", name="att_ps")
                for i in range(4):
                    nc.tensor.matmul(att_ps[:, :2 * H], atsb[:, i, :],
                                     av[:, i, :2 * H],
                                     start=(i == 0), stop=(i == 3))
                nc.vector.tensor_copy(pay[:, HID:HID + 2 * H],
                                      att_ps[:, :2 * H])
                nc.vector.memset(pay[:, HID + 2 * H:], 0.0)
                nc.sync.dma_start(ag_in[lyr][c * 128:(c + 1) * 128, :], pay[:])

            def allgather(lyr, half):
                if half == 0 or os.environ.get("KAG", "0") == "1":
                    return
                nc.gpsimd.collective_compute(
                    "AllGather", OP.bypass, replica_groups=RG,
                    ins=[ag_in[lyr].opt()], outs=[ag_out[lyr].opt()])

            def apply_w(msum, wtiles, wslice, out_ps, start, stop):
                """out_ps [128,512] (+)= msum [128,512] @ W via PE transpose.

                msum: [128, 512] bf16."""
                tps = pp.tile([128, 512], bf16, tag="big", name="tps")
                for i in range(4):
                    nc.tensor.transpose(tps[:, i * 128:(i + 1) * 128],
                                        msum[:, i * 128:(i + 1) * 128], identb)
                tsb = wp.tile([128, 4, 128], bf16, tag="tsb", name="tsb")
                nc.vector.tensor_copy(tsb[:], tps[:].rearrange("p (i k) -> p i k", i=4))
                for i in range(4):
                    nc.tensor.matmul(out_ps[:], tsb[:, i, :], wtiles[:, i, wslice],
                                     start=start and i == 0, stop=stop and i == 3)

            KBCAST = os.environ.get("KBCAST", "1") == "1"

            def edge_chunk(lyr, c, adg):
                """Edge phase for one chunk: per-head normalized msum tiles."""
                H = HEADS if lyr == 2 else 1
                msg = mp.tile([128, KT, PW], bf16, tag="msg", name="msg",
                              bufs=3)
                if KEDGE in (2, 3):
                    nc.vector.memset(msg[:], 0.0)
                else:
                    nc.gpsimd.dma_gather(
                        out_ap=msg[:], in_ap=ag_out[lyr][:],
                        idxs_ap=sidx[:, c, :], num_idxs=ESL, num_idxs_reg=ESL,
                        elem_size=PW, single_packet=False)
                stt = st_all[:, c]
                pte = wp.tile([128, KT, HEADS], f32, tag="pte", name="pte")
                nc.vector.tensor_tensor(
                    out=pte[:, :, :H], in0=msg[:, :, HID:HID + H],
                    in1=adg[:, c * KT:(c + 1) * KT, H:2 * H],
                    op=OP.add)
                nc.vector.scalar_tensor_tensor(
                    out=pte[:, :, :H], in0=pte[:, :, :H], scalar=SLOPE,
                    in1=pte[:, :, :H], op0=OP.mult, op1=OP.max)
                nc.scalar.activation(pte[:, :, :H], pte[:, :, :H], AF.Exp)
                def head_msum(h):
                    msum_ps = pp.tile([128, HID], f32, tag="big", name="msum_ps")
                    s_ps = psb.tile([128, 2], f32, tag="sm", name="s_ps")
                    Pall = None
                    if KBCAST:
                        Pall = wp.tile([128, KT, 128], bf16, tag="ptall",
                                       name="ptall")
                        nc.vector.tensor_tensor(
                            out=Pall[:], in0=stt,
                            in1=pte[:, :, h:h + 1].broadcast_to([128, KT, 128]),
                            op=OP.mult)
                    for b in range(KT):
                        if KBCAST:
                            P = Pall[:, b, :]
                        else:
                            Pt = wp.tile([128, 128], bf16, tag="ptile",
                                         name="ptile")
                            nc.vector.tensor_scalar_mul(
                                out=Pt[:], in0=stt[:, b],
                                scalar1=pte[:, b, h:h + 1])
                            P = Pt[:]
                        nc.tensor.matmul(msum_ps[:], P, msg[:, b, :HID],
                                         start=(b == 0), stop=(b == KT - 1))
                        nc.tensor.matmul(s_ps[:, :2], P, ones2[:],
                                         start=(b == 0), stop=(b == KT - 1))
                    rec = wp.tile([128, 1], f32, tag="rec", name="rec")
                    nc.vector.tensor_scalar_add(out=rec[:], in0=s_ps[:, 0:1],
                                                scalar1=EPS_SM)
                    nc.vector.reciprocal(rec[:], rec[:])
                    md = f32r if lyr == 0 else bf16
                    msum = wp.tile([128, HID], md, tag="msum", name="msum")
                    nc.vector.tensor_scalar_mul(out=msum[:], in0=msum_ps[:],
                                                scalar1=rec[:])
                    return msum
                return head_msum

            def layer_edge_to_out(lyr, gbias):
                """Edge phase + weight application; writes out rows to the
                SBUF-resident o_all and accumulates graph-norm partial sums;
                then AllReduces stats."""
                F = HEADS * HID if lyr == 2 else HID
                st1 = pp.tile([B, 512], f32, tag="big", name="st1")
                st2 = pp.tile([B, 512], f32, tag="big", name="st2")
                nslice = F // 512
                # one gather for all chunks' dst attention columns
                adg = mp.tile([128, CHUNKS * KT, AW], bf16, tag="adg",
                              name="adg", bufs=1)
                if KEDGE in (1, 3):
                    nc.vector.memset(adg[:], 0.0)
                else:
                    nc.gpsimd.dma_gather(
                        out_ap=adg[:], in_ap=ag_out[lyr][:, HID:],
                        idxs_ap=didx[:],
                        num_idxs=CHUNKS * ESL, num_idxs_reg=CHUNKS * ESL,
                        elem_size=AW, elem_step=PW,
                        single_packet=False)
                for c in range(CHUNKS):
                    head_msum = edge_chunk(lyr, c, adg)
                    H = HEADS if lyr == 2 else 1
                    for h in range(H):
                        msum = head_msum(h)
                        sl = slice(h * HID, (h + 1) * HID)
                        if lyr == 0:
                            nc.vector.tensor_tensor(
                                out=o_all[:, c, :HID], in0=msum[:].bitcast(f32),
                                in1=gbias[:, :HID], op=OP.add)
                        else:
                            wt = w2t if lyr == 1 else w3t
                            o_ps = pp.tile([128, HID], f32, tag="big",
                                           name="o_ps")
                            apply_w(msum, wt, sl, o_ps, True, True)
                            nc.vector.tensor_tensor(
                                out=o_all[:, c, sl], in0=o_ps[:],
                                in1=gbias[:, sl], op=OP.add)
                    for s in range(nslice):
                        sl = slice(s * 512, (s + 1) * 512)
                        sq = wp.tile([128, 512], bf16, tag="scratch", name="sq")
                        nc.vector.tensor_tensor(out=sq[:], in0=o_all[:, c, sl],
                                                in1=o_all[:, c, sl], op=OP.mult)
                        first = c == 0 and s == 0
                        last = c == CHUNKS - 1 and s == nslice - 1
                        nc.tensor.matmul(st1[:, :512], segmb[:, c, :],
                                         o_all[:, c, sl],
                                         start=first, stop=last)
                        nc.tensor.matmul(st2[:, :512], segmb[:, c, :], sq[:],
                                         start=first, stop=last)
                s12 = wp.tile([B, 4], f32, tag="s12", name="s12")
                nc.vector.memset(s12[:], 0.0)
                nc.vector.reduce_sum(s12[:, 0:1], st1[:], axis=mybir.AxisListType.X)
                nc.vector.reduce_sum(s12[:, 1:2], st2[:], axis=mybir.AxisListType.X)
                nc.sync.dma_start(ar_in[lyr][:], s12[:])
                if os.environ.get("KAR", "0") == "1":
                    nc.sync.dma_start(ar_out[lyr][:], s12[:])
                    return
                nc.gpsimd.collective_compute(
                    "AllReduce", OP.add, replica_groups=RG,
                    ins=[ar_in[lyr].opt()], outs=[ar_out[lyr].opt()])

            def layer_norm_consume(lyr, ggb, consume):
                """Normalize + gelu the SBUF-resident rows, hand tiles on."""
                F = HEADS * HID if lyr == 2 else HID
                ccol = 1 if lyr == 2 else 0
                sg = wp.tile([B, 4], f32, tag="s12", name="sg")
                nc.sync.dma_start(sg[:], ar_out[lyr][:])
                mu = wp.tile([B, 4], f32, tag="mu", name="mu")
                nc.vector.tensor_tensor(out=mu[:, 0:1], in0=sg[:, 0:1],
                                        in1=cinv[:, ccol:ccol + 1], op=OP.mult)
                nc.vector.tensor_tensor(out=mu[:, 2:3], in0=sg[:, 1:2],
                                        in1=cinv[:, ccol:ccol + 1], op=OP.mult)
                nc.vector.tensor_tensor(out=mu[:, 3:4], in0=mu[:, 0:1],
                                        in1=mu[:, 0:1], op=OP.mult)
                nc.vector.tensor_tensor(out=mu[:, 2:3], in0=mu[:, 2:3],
                                        in1=mu[:, 3:4], op=OP.subtract)
                nc.vector.tensor_scalar_add(out=mu[:, 2:3], in0=mu[:, 2:3],
                                            scalar1=EPS_LN)
                nc.scalar.activation(mu[:, 2:3], mu[:, 2:3], AF.Sqrt)
                nc.vector.reciprocal(mu[:, 1:2], mu[:, 2:3])
                stats2 = wp.tile([B, 2], f32r, tag="stats2", name="stats2")
                nc.vector.tensor_copy(stats2[:], mu[:, 0:2])
                for c in range(CHUNKS):
                    nst_ps = psb.tile([128, 2], f32, tag="sm", name="nst_ps")
                    nc.tensor.matmul(nst_ps[:], segmT[:, c, :], stats2[:],
                                     start=True, stop=True)
                    nst = wp.tile([128, 2], f32, tag="nst", name="nst")
                    nc.vector.tensor_copy(nst[:], nst_ps[:])
                    act = wp.tile([128, HEADS * HID], bf16, tag="actc",
                                  name="actc")
                    nc.vector.tensor_scalar(
                        out=act[:, :F], in0=o_all[:, c, :F],
                        scalar1=nst[:, 0:1], scalar2=nst[:, 1:2],
                        op0=OP.subtract, op1=OP.mult)
                    if not gb_trivial:
                        nc.vector.tensor_tensor(
                            out=act[:, :F], in0=act[:, :F],
                            in1=ggb[:, 0, :F], op=OP.mult)
                        nc.vector.tensor_tensor(
                            out=act[:, :F], in0=act[:, :F],
                            in1=ggb[:, 1, :F], op=OP.add)
                    nc.scalar.activation(act[:, :F], act[:, :F], AF.Gelu)
                    consume(c, act)

            # ================= layer 1 payload =================
            for c in range(CHUNKS):
                h1_ps = pp.tile([128, HID], f32, tag="big", name="h1_ps")
                av_ps = psb.tile([128, 2], f32, tag="sm", name="av_ps")
                xt_t = sp.tile([128, INP // 128, 128], bf16, tag="xt", name="xt")
                nc.sync.dma_start(xt_t[:], i_xt[c])
                for t in range(INP // 128):
                    nc.tensor.matmul(h1_ps[:], xt_t[:, t, :], w1t[:, t, :],
                                     start=(t == 0), stop=(t == INP // 128 - 1))
                    nc.tensor.matmul(av_ps[:], xt_t[:, t, :], w1avt[:, t, :],
                                     start=(t == 0), stop=(t == INP // 128 - 1))
                pay = sp.tile([128, PW], bf16, tag="pay", name="pay", bufs=2)
                nc.vector.tensor_copy(pay[:, :HID], h1_ps[:])
                nc.vector.tensor_copy(pay[:, HID:HID + 2], av_ps[:])
                nc.vector.memset(pay[:, HID + 2:], 0.0)
                nc.sync.dma_start(ag_in[0][c * 128:(c + 1) * 128, :], pay[:])
                if c == 3:
                    allgather(0, 0)
            allgather(0, 1)
            load_statics()

            # ================= layers =================
            if STAGE >= 1:
                gb1 = load_gbias(0)
                ggb1 = load_ggb(0)
                av1 = load_av(1)
                layer_edge_to_out(0, gb1)
            if STAGE >= 2:
                gb2 = load_gbias(1)
                def consume1(c, act):
                    attn_payload(1, c, act, av1)
                    if c == 3:
                        allgather(1, 0)
                layer_norm_consume(0, ggb1, consume1)
                allgather(1, 1)
            if STAGE >= 3:
                ggb2 = load_ggb(1)
                av2 = load_av(2)
                layer_edge_to_out(1, gb2)
                gb3 = load_gbias(2)
                def consume2(c, act):
                    attn_payload(2, c, act, av2)
                    if c == 3:
                        allgather(2, 0)
                layer_norm_consume(1, ggb2, consume2)
                allgather(2, 1)
            if STAGE >= 4:
                ggb3 = load_ggb(2)
                layer_edge_to_out(2, gb3)
                pool_ps = [pp.tile([B, 512], f32, tag="big", name=f"pool{s}")
                           for s in range(3)]

                def pool_consume(c, act):
                    for s in range(3):
                        nc.tensor.matmul(pool_ps[s][:], segmb[:, c, :],
                                         act[:, s * 512:(s + 1) * 512],
                                         start=(c == 0), stop=(c == CHUNKS - 1))

                layer_norm_consume(2, ggb3, pool_consume)

            if STAGE < 4:
                fin0 = wp.tile([B, OUT], f32, tag="scratch", name="fin0")
                nc.vector.memset(fin0[:], 0.0)
                nc.sync.dma_start(o_out[:], fin0[:])
            # ================= pooling + MLP =================
            if STAGE >= 4:
                pool_sb = wp.tile([B, HEADS * HID], f32, tag="outc", name="poolsb")
                for s in range(3):
                    nc.vector.tensor_copy(pool_sb[:, s * 512:(s + 1) * 512],
                                          pool_ps[s][:])
                nc.sync.dma_start(arp_in[:], pool_sb[:])
                nc.gpsimd.collective_compute(
                    "AllReduce", OP.add, replica_groups=RG,
                    ins=[arp_in.opt()], outs=[arp_out.opt()])
                pooled = wp.tile([B, HEADS * HID], f32r, tag="outc", name="pooled")
                nc.sync.dma_start(pooled[:], arp_out[:].bitcast(f32r))
                nc.vector.tensor_scalar_mul(out=pooled[:], in0=pooled[:].bitcast(f32),
                                            scalar1=cinv[:, 2:3])

                # MLP weights reuse the w1t slot
                rwt = wb.tile([128, 12, HID], bf16, tag="wbig", name="rwt")
                nc.gpsimd.dma_start(rwt[:], i_rw)
                l1wt = cp.tile([128, 4, HID // 2], bf16, tag="wshare", name="l1wt")
                nc.gpsimd.dma_start(l1wt[:], i_l1w)
                l2wt = cp.tile([128, 2, HID // 4], bf16)
                nc.gpsimd.dma_start(l2wt[:], i_l2w)
                owt = cp.tile([128, OUT], bf16)
                nc.gpsimd.dma_start(owt[:], i_ow)

                pT = wp.tile([128, 12, B], bf16, tag="msum", name="pT")
                for i in range(12):
                    ps = psb.tile([128, B], f32r, tag="sm", name="pT_ps")
                    nc.tensor.transpose(ps[:], pooled[:, i * 128:(i + 1) * 128],
                                        ident[:B, :B])
                    nc.vector.tensor_copy(pT[:, i, :], ps[:])

                def mlp_layer(in_t, nkt, wtiles, nm, brow, actf, nm_tag):
                    outt = wp.tile([128, nm, B], bf16, tag=nm_tag, name=nm_tag)
                    for m in range(nm):
                        ps = psb.tile([128, B], f32, tag="sm", name="mlp_ps")
                        for t in range(nkt):
                            nc.tensor.matmul(ps[:], wtiles[:, t, m * 128:(m + 1) * 128],
                                             in_t[:, t, :],
                                             start=(t == 0), stop=(t == nkt - 1))
                        nc.scalar.activation(outt[:, m, :], ps[:], actf,
                                             bias=brow[:, m:m + 1])
                    return outt

                m1 = mlp_layer(pT, 12, rwt, 4, mlpb["rbp"], AF.Gelu, "mlp1")
                m2 = mlp_layer(m1, 4, l1wt, 2, mlpb["l1bp"], AF.Gelu, "mlp2")
                m3 = mlp_layer(m2, 2, l2wt, 1, mlpb["l2bp"], AF.Gelu, "mlp3")
                mf = wp.tile([128, 2, B], bf16, tag="mlp1", name="mf")
                for m in range(2):
                    ps = psb.tile([128, B], f32, tag="sm", name="mf_ps")
                    nc.tensor.matmul(ps[:], owt[:, m * 128:(m + 1) * 128], m3[:, 0, :],
                                     start=True, stop=True)
                    nc.scalar.activation(mf[:, m, :], ps[:], AF.Identity,
                                         bias=mlpb["obp"][:, m:m + 1])
                fin = wp.tile([B, OUT], f32, tag="finout", name="fin")
                for m in range(2):
                    ps = psb.tile([B, 128], bf16, tag="sm", name="fin_ps")
                    nc.tensor.transpose(ps[:], mf[:, m, :], identb)
                    nc.vector.tensor_copy(fin[:, m * 128:(m + 1) * 128], ps[:])
                nc.sync.dma_start(o_out[:], fin[:])

    nc.compile()
    return nc


_CACHE = {}


def kernel(**inputs):
    key, in_maps = preprocess(inputs)
    if key not in _CACHE:
        _CACHE[key] = build(key)
    nc = _CACHE[key]
    res = run_bass_kernel_spmd(nc, in_maps, core_ids=list(range(NCORES)))
    return res.results[0]["out"].astype(np.float32)
